# revision 1
# baseline (speedup 1.0000x reference)
"""Autoformer attention block kernel for 8 TRN2 NeuronCores.

Math reduction (validated vs reference to 2e-7):
 - output = x + AutoCorrelation(series_decomp(LN(x)))  (final decomp s2+t2 == x2)
 - mean over lags of the FFT cross-correlation == (sum_t Q)*(sum_t K)  (DC bin),
   so no FFT is needed: top-k stats come from column sums of `seasonal`.
 - beta cancels exactly (band operator has row-sum 1); gamma folds into
   Wvo = diag(gamma) @ Wv @ Wo and the qsum scaling.
 - delay aggregation = 64-tap circular FIR along time with data-dependent
   weights -> banded Toeplitz matmul on the TensorEngine.

Sharding: data-parallel over batch (B=8 -> 8 cores); one [64]-float AllReduce
for the global top-40 channel selection.
"""

import sys

if "/opt/trn_rl_repo" not in sys.path:
    sys.path.insert(0, "/opt/trn_rl_repo")

import numpy as np

L = 3072
D = 512
NT = L // 128  # 24 time tiles
H = 8
DK = 64
KTOP = 40
PAD = 12  # (25-1)//2
EPS = 1e-5
NCORES = 8
HL = float(H * L)

_CACHE = {}


def _np_consts():
    t = np.arange(L)
    lo = np.maximum(t - PAD, 0)
    hi = np.minimum(t + PAD + 1, L)
    inv = 1.0 / (hi - lo).astype(np.float64)

    # phi[s] = 1 - sum over t in the window around s of 1/win(t); nonzero only
    # in the first/last 24 positions.
    phi = np.ones(L, np.float64)
    for s in range(L):
        a = max(0, s - PAD)
        b = min(L, s + PAD + 1)
        phi[s] -= inv[a:b].sum()

    # band lhsT consts, all [128,128], K = a full z tile, zero-padded:
    # chunk X in {A: s = t0-128+j, B: s = t0+j, C: s = t0+128+j}:
    #   M[j, p] = delta(s, t0+p) - [|t0+p - s| <= PAD] / win(t0+p)
    def band(t0, soff):
        j = np.arange(128)[:, None]
        p = np.arange(128)[None, :]
        s = soff + j
        tp = t0 + p
        m = (np.abs(tp - s) <= PAD) & (s >= 0) & (s < L)
        M = -(m * inv[np.clip(tp, 0, L - 1)])
        M = M + (s == tp) * 1.0
        return np.ascontiguousarray(M, np.float32)

    t0m = 1280  # any interior tile
    b_A = band(t0m, t0m - 128)
    b_C = band(t0m, t0m + 128)
    b_Bf = band(0, 0)
    b_Bm = band(t0m, t0m)
    b_Bl = band(L - 128, L - 128)
    phi48 = np.zeros((128, 1), np.float32)
    phi48[:24, 0] = phi[:24]
    phi48[64:88, 0] = phi[-24:]
    ident = np.eye(128, dtype=np.float32)
    return b_A, b_C, b_Bf, b_Bm, b_Bl, phi48, ident


def _build():
    import concourse.bass as bass
    import concourse.tile as tile
    import concourse.mybir as mybir
    from concourse import bacc
    import bass_rust
    import ml_dtypes

    dt = mybir.dt
    f32 = dt.float32
    bf16 = dt.bfloat16
    AF = mybir.ActivationFunctionType
    ALU = mybir.AluOpType
    AX = mybir.AxisListType
    ts = bass.ts

    nc = bacc.Bacc(None, target_bir_lowering=False)

    xe = nc.dram_tensor("xb", [L, D], f32, kind="ExternalInput")
    wqe = nc.dram_tensor("Wq", [D, D], f32, kind="ExternalInput")
    wke = nc.dram_tensor("Wk", [D, D], f32, kind="ExternalInput")
    wve = nc.dram_tensor("Wv", [D, D], f32, kind="ExternalInput")
    woe = nc.dram_tensor("Wo", [D, D], f32, kind="ExternalInput")
    bqe = nc.dram_tensor("bq", [D], f32, kind="ExternalInput")
    bke = nc.dram_tensor("bk", [D], f32, kind="ExternalInput")
    bve = nc.dram_tensor("bv", [D], f32, kind="ExternalInput")
    boe = nc.dram_tensor("bo", [D], f32, kind="ExternalInput")
    gme = nc.dram_tensor("gamma", [D], f32, kind="ExternalInput")
    oute = nc.dram_tensor("out", [L, D], f32, kind="ExternalOutput")

    bA, bC, bBf, bBm, bBl, phi48, ident = _np_consts()
    bf = ml_dtypes.bfloat16
    cbA = nc.inline_tensor(bA.astype(bf), "c_bA")
    cbC = nc.inline_tensor(bC.astype(bf), "c_bC")
    cbBf = nc.inline_tensor(bBf.astype(bf), "c_bBf")
    cbBm = nc.inline_tensor(bBm.astype(bf), "c_bBm")
    cbBl = nc.inline_tensor(bBl.astype(bf), "c_bBl")
    cphi = nc.inline_tensor(phi48.astype(bf), "c_phi")
    cid = nc.inline_tensor(ident.astype(bf), "c_id")
    cones1x64 = nc.inline_tensor(np.ones((1, 64), np.float32), "c_o64")
    cones1x128b = nc.inline_tensor(np.ones((1, 128), bf), "c_o128b")

    from contextlib import ExitStack

    with tile.TileContext(nc) as tc, ExitStack() as ctx:
        pc = ctx.enter_context(tc.tile_pool(name="consts", bufs=1))
        px = ctx.enter_context(tc.tile_pool(name="xarr", bufs=NT))
        pz = ctx.enter_context(tc.tile_pool(name="zroll", bufs=10))
        pvo = ctx.enter_context(tc.tile_pool(name="voarr", bufs=NT))
        pwvo = ctx.enter_context(tc.tile_pool(name="wvo", bufs=4))
        pwt = ctx.enter_context(tc.tile_pool(name="wtmp", bufs=4))
        pwork = ctx.enter_context(tc.tile_pool(name="work", bufs=3))
        psq = ctx.enter_context(tc.tile_pool(name="sqscr", bufs=2))
        pstt = ctx.enter_context(tc.tile_pool(name="stats", bufs=3))
        psm = ctx.enter_context(tc.tile_pool(name="smalls", bufs=2))
        pout = ctx.enter_context(tc.tile_pool(name="osb", bufs=3))
        pseasT = ctx.enter_context(tc.tile_pool(name="seasT", bufs=3))
        pdram = ctx.enter_context(tc.tile_pool(name="dram", bufs=1, space="DRAM"))
        qst = ctx.enter_context(tc.tile_pool(name="ps_st", bufs=1, space="PSUM"))
        qtp = ctx.enter_context(tc.tile_pool(name="ps_tp", bufs=1, space="PSUM"))
        qvo = ctx.enter_context(tc.tile_pool(name="ps_vo", bufs=2, space="PSUM"))
        qsm = ctx.enter_context(tc.tile_pool(name="ps_sm", bufs=2, space="PSUM"))
        qtap = ctx.enter_context(tc.tile_pool(name="ps_tap", bufs=2, space="PSUM"))

        # ---------------- constants to SBUF ----------------
        def cload(name, shape, src, dtype=f32):
            t = pc.tile(list(shape), dtype, tag=name)
            nc.sync.dma_start(t[:], src)
            return t

        idt = cload("idt", (128, 128), cid[:, :], bf16)
        gammaP = pc.tile([128, 4], f32, tag="gammaP")
        nc.sync.dma_start(gammaP[:], gme[:].rearrange("(a b) -> b a", b=128))
        bndA = cload("bndA", (128, 128), cbA[:, :], bf16)
        bndC = cload("bndC", (128, 128), cbC[:, :], bf16)
        bndBf = cload("bndBf", (128, 128), cbBf[:, :], bf16)
        bndBm = cload("bndBm", (128, 128), cbBm[:, :], bf16)
        bndBl = cload("bndBl", (128, 128), cbBl[:, :], bf16)
        phis = cload("phis", (128, 1), cphi[:, :], bf16)
        o1x64 = cload("o1x64", (1, 64), cones1x64[:, :])
        o1x128b = cload("o1x128b", (1, 128), cones1x128b[:, :], bf16)
        bvP = pc.tile([128, 4], f32, tag="bvP")
        nc.sync.dma_start(bvP[:], bve[:].rearrange("(a b) -> b a", b=128))
        bqv = pc.tile([1, 512], f32, tag="bqv")
        nc.sync.dma_start(bqv[:], bqe[:])
        bkv = pc.tile([1, 512], f32, tag="bkv")
        nc.sync.dma_start(bkv[:], bke[:])
        bov = pc.tile([1, 512], f32, tag="bov")
        nc.sync.dma_start(bov[:], boe[:])
        bq_sc = pc.tile([1, 512], f32, tag="bq_sc")
        nc.scalar.mul(bq_sc[:], bqv[:], float(L))
        bk_sc = pc.tile([1, 512], f32, tag="bk_sc")
        nc.scalar.mul(bk_sc[:], bkv[:], float(L))

        ones64 = nc.const_aps.tensor(1.0, (64, 1))

        # toeplitz scratch in DRAM ([128 x 192] p-major), zeroed early
        toep2d = pdram.tile([128, 192], bf16, tag="toep2d")
        zline = pc.tile([128, 192], bf16, tag="zline")
        nc.vector.memset(zline[:], 0.0)
        nc.sync.dma_start(toep2d[:], zline[:])
        wfd = pdram.tile([64], bf16, tag="wfd")

        # ---------------- x tiles + grouped LN stats ----------------
        xt = [None] * NT
        zt = [None] * NT

        def emit_group(tiles):
            st = pstt.tile([128, 36], f32, tag="st")
            n = len(tiles)
            for j, i in enumerate(tiles):
                x = px.tile([128, 512], f32, tag="x")
                nc.sync.dma_start(x[:], xe[ts(i, 128), :])
                xt[i] = x
                nc.vector.tensor_reduce(
                    st[:, j : j + 1], x[:], axis=AX.X, op=ALU.add
                )
                sq = psq.tile([128, 512], f32, tag="sq")
                nc.scalar.activation(
                    sq[:], x[:], AF.Square, accum_out=st[:, 4 + j : 5 + j]
                )
            # mu = sx/D; mu2 = mu^2; t2 = sxx/D + eps; var = t2 - mu2
            # sd = sqrt(var); r = 1/sd; nmr = -(mu*r)
            nc.vector.tensor_scalar(
                st[:, 8 : 8 + n], st[:, 0:n], 1.0 / D, None, op0=ALU.mult
            )
            nc.vector.tensor_tensor(
                st[:, 12 : 12 + n], st[:, 8 : 8 + n], st[:, 8 : 8 + n], op=ALU.mult
            )
            nc.vector.tensor_scalar(
                st[:, 16 : 16 + n], st[:, 4 : 4 + n], 1.0 / D, EPS,
                op0=ALU.mult, op1=ALU.add,
            )
            nc.vector.tensor_tensor(
                st[:, 20 : 20 + n], st[:, 16 : 16 + n], st[:, 12 : 12 + n],
                op=ALU.subtract,
            )
            nc.scalar.activation(st[:, 24 : 24 + n], st[:, 20 : 20 + n], AF.Sqrt)
            nc.vector.reciprocal(st[:, 28 : 28 + n], st[:, 24 : 24 + n])
            nc.vector.tensor_tensor(
                st[:, 32 : 32 + n], st[:, 8 : 8 + n], st[:, 28 : 28 + n],
                op=ALU.mult,
            )
            nc.vector.tensor_scalar(
                st[:, 32 : 32 + n], st[:, 32 : 32 + n], -1.0, None, op0=ALU.mult
            )
            for j, i in enumerate(tiles):
                z = pz.tile([128, 512], bf16, tag="z")
                nc.scalar.activation(
                    z[:], xt[i][:], AF.Identity,
                    bias=st[:, 32 + j : 33 + j], scale=st[:, 28 + j : 29 + j],
                )
                zt[i] = z

        # ---------------- seasonal (banded matmul) + vo ----------------
        vo = [None] * NT
        wvo = []  # filled by weight prep below
        toep_ref = {}

        def emit_seasonal(i):
            sps = qst.tile([128, 512], f32)
            if i == 0:
                chunks = [(bndBf[:], zt[0][:, :]), (bndC[:], zt[1][:, :])]
            elif i == NT - 1:
                chunks = [(bndA[:], zt[22][:, :]), (bndBl[:], zt[23][:, :])]
            else:
                chunks = [
                    (bndA[:], zt[i - 1][:, :]),
                    (bndBm[:], zt[i][:, :]),
                    (bndC[:], zt[i + 1][:, :]),
                ]
            nck = len(chunks)
            for k, (lt, rz) in enumerate(chunks):
                nc.tensor.matmul(
                    sps[:], lt, rz, start=(k == 0), stop=(k == nck - 1)
                )
            seas = pwork.tile([128, 512], bf16, tag="seas")
            nc.scalar.copy(seas[:], sps[:])
            tp = qtp.tile([128, 512], bf16, tag="tp")
            for c in range(4):
                nc.tensor.transpose(tp[:, ts(c, 128)], seas[:, ts(c, 128)], idt[:])
            sT = pseasT.tile([128, 512], bf16, tag="sT")
            nc.vector.tensor_copy(sT[:], tp[:])
            vps = qvo.tile([128, 512], f32)
            for c in range(4):
                nc.tensor.matmul(
                    vps[:], sT[:, ts(c, 128)], wvo[c][:],
                    start=(c == 0), stop=(c == 3),
                )
            v = pvo.tile([128, 512], bf16, tag="vo")
            nc.scalar.copy(v[:], vps[:])
            vo[i] = v

        # ---------------- tap + residual + output ----------------
        def emit_tap(i):
            toepA = toep_ref["A"]
            toepB = toep_ref["B"]
            cvb = toep_ref["cvb"]
            tps = qtap.tile([128, 512], f32)
            nc.tensor.matmul(tps[:], toepA[:], vo[i][:], start=True, stop=False)
            nc.tensor.matmul(
                tps[:], toepB[:], vo[(i + 1) % NT][0:63, :],
                start=False, stop=False,
            )
            nc.tensor.matmul(tps[:], o1x128b[:], cvb[:], start=False, stop=True)
            osb = pout.tile([128, 512], f32, tag="osb")
            nc.vector.tensor_tensor(osb[:], xt[i][:], tps[:], op=ALU.add)
            if i % 2 == 0:
                nc.scalar.dma_start(oute[ts(i, 128), :], osb[:])
            else:
                nc.sync.dma_start(oute[ts(i, 128), :], osb[:])

        # ---------------- early qsum from the 48 boundary rows ----------------
        # phi is nonzero only on rows [0:24) and [L-24:L); LN is row-wise, so
        # compute z for just those rows in a dedicated tile (head at partition
        # 0, tail at partition 64 to satisfy matmul base-partition rules) and
        # feed the collective ~15us earlier than the full tiles would.
        with tc.high_priority():
            x48 = pwork.tile([128, 512], f32, tag="x48")
            nc.sync.dma_start(x48[0:24, :], xe[0:24, :])
            nc.sync.dma_start(x48[64:88, :], xe[L - 24 : L, :])
            st8 = pstt.tile([128, 36], f32, tag="st")
            nc.vector.tensor_reduce(
                st8[0:88, 0:1], x48[0:88, :], axis=AX.X, op=ALU.add
            )
            sq8 = psq.tile([128, 512], f32, tag="sq")
            nc.scalar.activation(
                sq8[0:88, :], x48[0:88, :], AF.Square,
                accum_out=st8[0:88, 1:2],
            )
            nc.vector.tensor_scalar(
                st8[0:88, 2:3], st8[0:88, 0:1], 1.0 / D, None, op0=ALU.mult
            )
            nc.vector.tensor_tensor(
                st8[0:88, 3:4], st8[0:88, 2:3], st8[0:88, 2:3], op=ALU.mult
            )
            nc.vector.tensor_scalar(
                st8[0:88, 4:5], st8[0:88, 1:2], 1.0 / D, EPS,
                op0=ALU.mult, op1=ALU.add,
            )
            nc.vector.tensor_tensor(
                st8[0:88, 5:6], st8[0:88, 4:5], st8[0:88, 3:4], op=ALU.subtract
            )
            nc.scalar.activation(st8[0:88, 6:7], st8[0:88, 5:6], AF.Sqrt)
            nc.vector.reciprocal(st8[0:88, 7:8], st8[0:88, 6:7])
            nc.vector.tensor_tensor(
                st8[0:88, 8:9], st8[0:88, 2:3], st8[0:88, 7:8], op=ALU.mult
            )
            nc.vector.tensor_scalar(
                st8[0:88, 9:10], st8[0:88, 8:9], -1.0, None, op0=ALU.mult
            )
            z48 = pwork.tile([128, 512], bf16, tag="z48")
            nc.scalar.activation(
                z48[0:88, :], x48[0:88, :], AF.Identity,
                bias=st8[0:88, 9:10], scale=st8[0:88, 7:8],
            )
            qps = qsm.tile([128, 8], f32, tag="sm")
            for c in range(4):
                nc.tensor.matmul(
                    qps[:, c : c + 1], z48[0:24, ts(c, 128)], phis[0:24, :],
                    start=True, stop=True,
                )
            for c in range(4):
                nc.tensor.matmul(
                    qps[:, 4 + c : 5 + c], z48[64:88, ts(c, 128)],
                    phis[64:88, :], start=True, stop=True,
                )

        # ---------------- weight prep: Wvo = diag(gamma) Wv Wo, cvec ----------------
        wo_sb = []
        for a in range(4):
            w = pwt.tile([128, 512], f32, tag="wo")
            nc.sync.dma_start(w[:], woe[ts(a, 128), :])
            wo_sb.append(w)
        wob = []
        for a in range(4):
            w = pwt.tile([128, 512], bf16, tag="wob")
            nc.vector.tensor_copy(w[:], wo_sb[a][:])
            wob.append(w)
        wv_sc = []
        for a in range(4):
            w = pwt.tile([128, 512], f32, tag="wv")
            nc.sync.dma_start(w[:], wve[ts(a, 128), :])
            ws = pwt.tile([128, 512], bf16, tag="wvs")
            nc.scalar.activation(ws[:], w[:], AF.Identity, scale=gammaP[:, a : a + 1])
            wv_sc.append(ws)
        wvT = []
        for c in range(4):
            w = pwt.tile([128, 512], bf16, tag="wvT")
            wvT.append(w)
        for a in range(4):
            for c in range(4):
                tp = qtp.tile([128, 128], bf16, tag="tp")
                nc.tensor.transpose(tp[:], wv_sc[a][:, ts(c, 128)], idt[:])
                nc.vector.tensor_copy(wvT[c][:, ts(a, 128)], tp[:])
        for a in range(4):
            vps = qvo.tile([128, 512], f32)
            for c in range(4):
                nc.tensor.matmul(
                    vps[:], wvT[c][:, ts(a, 128)], wob[c][:],
                    start=(c == 0), stop=(c == 3),
                )
            w = pwvo.tile([128, 512], bf16, tag="wvo")
            nc.scalar.copy(w[:], vps[:])
            wvo.append(w)

        # cvec = bv @ Wo + bo (bf16 row for the tap matmul)
        cps = qsm.tile([1, 512], f32, tag="sm")
        for c in range(4):
            nc.tensor.matmul(
                cps[:], bvP[:, c : c + 1], wo_sb[c][:],
                start=(c == 0), stop=(c == 3),
            )
        cv_sb = psm.tile([1, 512], f32, tag="cv")
        nc.vector.tensor_tensor(cv_sb[:], cps[:], bov[:], op=ALU.add)
        cvb = psm.tile([1, 512], bf16, tag="cvb")
        nc.vector.tensor_copy(cvb[:], cv_sb[:])
        toep_ref["cvb"] = cvb

        # ---------------- mv -> collective -> softmax weights -> toeplitz ----------------
        with tc.high_priority():
            wq_sb = []
            wk_sb = []
            for a in range(4):
                w = pwt.tile([128, 512], f32, tag="wq")
                nc.sync.dma_start(w[:], wqe[ts(a, 128), :])
                wb = pwt.tile([128, 512], bf16, tag="wqb")
                nc.vector.tensor_copy(wb[:], w[:])
                wq_sb.append(wb)
                w = pwt.tile([128, 512], f32, tag="wk")
                nc.sync.dma_start(w[:], wke[ts(a, 128), :])
                wb = pwt.tile([128, 512], bf16, tag="wkb")
                nc.vector.tensor_copy(wb[:], w[:])
                wk_sb.append(wb)

            qs_t = psm.tile([128, 4], f32, tag="qst")
            nc.scalar.copy(qs_t[:], qps[:, 4:8])
            qs_g = psm.tile([128, 4], bf16, tag="qsg")
            nc.vector.tensor_tensor(qs_g[:], qps[:, 0:4], qs_t[:], op=ALU.add)
            gb = psm.tile([128, 4], bf16, tag="gb")
            nc.vector.tensor_copy(gb[:], gammaP[:])
            nc.vector.tensor_tensor(qs_g[:], qs_g[:], gb[:], op=ALU.mult)

            qs_ps = qsm.tile([1, 512], f32, tag="sm")
            for c in range(4):
                nc.tensor.matmul(
                    qs_ps[:], qs_g[:, c : c + 1], wq_sb[c][:],
                    start=(c == 0), stop=(c == 3),
                )
            qsv = psm.tile([1, 512], f32, tag="qsv")
            nc.vector.tensor_tensor(qsv[:], qs_ps[:], bq_sc[:], op=ALU.add)
            ks_ps = qsm.tile([1, 512], f32, tag="sm")
            for c in range(4):
                nc.tensor.matmul(
                    ks_ps[:], qs_g[:, c : c + 1], wk_sb[c][:],
                    start=(c == 0), stop=(c == 3),
                )
            ksv = psm.tile([1, 512], f32, tag="ksv")
            nc.vector.tensor_tensor(ksv[:], ks_ps[:], bk_sc[:], op=ALU.add)

            pr = psm.tile([1, 512], f32, tag="pr")
            nc.vector.tensor_tensor(pr[:], qsv[:], ksv[:], op=ALU.mult)
            mvr = psm.tile([1, 64], f32, tag="mvr")
            nc.vector.tensor_reduce(
                mvr[:], pr[:].rearrange("p (h c) -> p c h", h=H),
                axis=AX.X, op=ALU.add,
            )
            mv = psm.tile([1, 64], f32, tag="mv")
            nc.scalar.mul(mv[:], mvr[:], 1.0 / HL)

            ccin = pdram.tile([64], f32, tag="ccin")
            ccout = pdram.tile([8, 64], f32, tag="ccout")
            mvd = pdram.tile([64], f32, tag="mvd")
            nc.gpsimd.dma_start(ccin[:], mv[:])
            nc.gpsimd.collective_compute(
                "AllGather",
                ALU.bypass,
                replica_groups=[list(range(NCORES))],
                ins=[ccin[:].opt()],
                outs=[ccout[:].opt()],
            )

        # ---------------- main pipeline ----------------
        groups = [[0, 1, 2, 3], [4, 5, 6, 7], [8, 9, 10, 11],
                  [12, 13, 14, 15], [16, 17, 18, 19]]
        state = {"sea": 0, "tap": 0}

        def advance():
            while state["sea"] < NT:
                i = state["sea"]
                need = [n for n in (i - 1, i, i + 1) if 0 <= n < NT]
                if not all(zt[n] is not None for n in need):
                    break
                emit_seasonal(i)
                state["sea"] += 1

        for g in groups:
            emit_group(g)
            advance()
        # post-collective: global mask, softmax weights, toeplitz build
        nc.gpsimd.dma_start(mvd[:], mv[:])
        g8 = psm.tile([1, 512], f32, tag="g8")
        nc.sync.dma_start(g8[:], ccout[:])
        g_row = psm.tile([1, 64], f32, tag="grow")
        nc.vector.tensor_reduce(
            g_row[:], g8[:].rearrange("p (r c) -> p c r", r=NCORES),
            axis=AX.X, op=ALU.add,
        )
        gP_ps = qsm.tile([64, 1], f32, tag="sm")
        nc.tensor.matmul(
            gP_ps[:], g_row[:], nc.const_aps.tensor(1.0, (1, 1)),
            start=True, stop=True,
        )
        gP = psm.tile([64, 1], f32, tag="gP")
        nc.vector.tensor_copy(gP[:], gP_ps[:])
        mvP = psm.tile([64, 1], f32, tag="mvP")
        nc.sync.dma_start(mvP[:], mvd[:])

        gf_ps = qsm.tile([64, 64], f32, tag="sm")
        nc.tensor.matmul(gf_ps[:], o1x64[:], g_row[:], start=True, stop=True)
        sc = psm.tile([64, 8], f32, tag="scm")
        cmp = psm.tile([64, 64], f32, tag="cmp")
        nc.vector.tensor_tensor(
            cmp[:], gf_ps[:], gP[:].to_broadcast((64, 64)), op=ALU.is_gt
        )
        nc.vector.tensor_reduce(sc[:, 0:1], cmp[:], axis=AX.X, op=ALU.add)
        nc.vector.tensor_scalar(
            sc[:, 1:2], sc[:, 0:1], KTOP - 0.5, None, op0=ALU.is_lt
        )
        nc.scalar.activation(sc[:, 2:3], mvP[:], AF.Exp)
        nc.vector.tensor_tensor(sc[:, 3:4], sc[:, 2:3], sc[:, 1:2], op=ALU.mult)
        s_ps = qsm.tile([1, 1], f32, tag="sm")
        nc.tensor.matmul(s_ps[:], sc[:, 3:4], ones64, start=True, stop=True)
        rs = psm.tile([1, 1], f32, tag="rs")
        nc.vector.reciprocal(rs[:], s_ps[:])
        rsf_ps = qsm.tile([64, 1], f32, tag="sm")
        nc.tensor.matmul(rsf_ps[:], o1x64[:], rs[:], start=True, stop=True)
        wf = psm.tile([64, 1], f32, tag="wf")
        nc.vector.tensor_tensor(wf[:], sc[:, 3:4], rsf_ps[:], op=ALU.mult)
        wfb = psm.tile([64, 1], bf16, tag="wfb")
        nc.vector.tensor_copy(wfb[:], wf[:])

        # toeplitz build: wf -> DRAM -> burst scatter (64-elem runs) ->
        # [p, j] tile -> PE transpose -> lhsT tiles toepA/toepB
        nc.sync.dma_start(wfd[:], wfb[:])
        dst = toep2d[:].flatten()
        dst.ap = bass_rust.VecI64Pair([[193, 128], [1, 64]])
        src = wfd[:].flatten()
        src.ap = bass_rust.VecI64Pair([[0, 128], [1, 64]])
        nc.sync.dma_start(dst, src)
        t2sb = pc.tile([128, 192], bf16, tag="t2sb")
        nc.sync.dma_start(t2sb[:], toep2d[:])
        tpa = qtp.tile([128, 128], bf16, tag="tp")
        nc.tensor.transpose(tpa[:], t2sb[:, 0:128], idt[:])
        toepA = pc.tile([128, 128], bf16, tag="toepA")
        nc.vector.tensor_copy(toepA[:], tpa[:])
        tpb = qtp.tile([128, 128], bf16, tag="tp")
        nc.tensor.transpose(tpb[0:63, :], t2sb[:, 128:191], idt[:])
        toepB = pc.tile([63, 128], bf16, tag="toepB")
        nc.vector.tensor_copy(toepB[:], tpb[0:63, :])
        toep_ref["A"] = toepA
        toep_ref["B"] = toepB
        # taps for the already-computed tiles; by the time the PE queue
        # reaches these, the collective (started ~17us) has completed, so
        # they do not jam the in-order engine queues.
        for i in range(17):
            emit_tap(i)
        emit_group([20, 21, 22, 23])
        advance()
        for i in range(17, NT):
            emit_tap(i)

    nc.finalize()
    return nc


def _get_nc():
    if "nc" not in _CACHE:
        _CACHE["nc"] = _build()
    return _CACHE["nc"]


def kernel_ext(inputs, trace=False):
    from concourse.bass_utils import run_bass_kernel_spmd

    nc = _get_nc()
    x = np.ascontiguousarray(inputs["x"], np.float32)
    common = {
        k: np.ascontiguousarray(inputs[k], np.float32)
        for k in ["Wq", "Wk", "Wv", "Wo", "bq", "bk", "bv", "bo", "gamma"]
    }
    in_maps = [{"xb": x[i], **common} for i in range(NCORES)]
    res = run_bass_kernel_spmd(nc, in_maps, list(range(NCORES)), trace=trace)
    out = np.stack([res.results[i]["out"] for i in range(NCORES)], axis=0)
    return out, res


def kernel(**inputs):
    out, _ = kernel_ext(inputs)
    return out



# revision 15
# speedup vs baseline: 1.0847x; 1.0847x over previous
"""Autoformer attention block kernel for 8 TRN2 NeuronCores.

Math reduction (validated vs reference to 1.3e-7 in numpy):
 - output = x + AutoCorrelation(series_decomp(LN(x)))  (final decomp s2+t2 == x2)
 - mean over lags of the FFT cross-correlation == (sum_t Q)*(sum_t K)  (DC bin),
   so no FFT is needed: top-k stats come from column sums of `seasonal`
   (nonzero only on the 48 boundary rows via the phi trick).
 - the series-decomp band (I - MA25) and the 64-tap delay FIR commute with
   the d-axis projection, so they FUSE into one 88-tap circular FIR C applied
   to y = LN(x) @ Wvo, with a rank-24 boundary correction for the clamped
   moving-average windows:  out = x + C(y) + corr + (bv@Wo + bo).
 - Wvo = diag(gamma) @ Wv @ Wo; beta drops (C and corr have row-sum 0 paths).

Schedule: all 24 tiles of LN -> transpose -> y = z@Wvo run before the
collective result is needed; the data-dependent part is only the toeplitz
build + 3 tap matmuls per tile, keeping the PE continuously busy (p-state).

Sharding: data-parallel over batch (B=8 -> 8 cores); one [64]-float AllGather
for the global top-40 channel selection.
"""

import sys

if "/opt/trn_rl_repo" not in sys.path:
    sys.path.insert(0, "/opt/trn_rl_repo")

import numpy as np

L = 3072
D = 512
NT = 24   # 128-row time tiles
NP = 12   # 256-row tile pairs
H = 8
DK = 64
KTOP = 40
PAD = 12
EPS = 1e-5
NCORES = 8
HL = float(H * L)

_CACHE = {}


def _np_consts():
    t = np.arange(L)
    lo = np.maximum(t - PAD, 0)
    hi = np.minimum(t + PAD + 1, L)
    inv = 1.0 / (hi - lo).astype(np.float64)

    phi = np.ones(L, np.float64)
    for s in range(L):
        a = max(0, s - PAD)
        b = min(L, s + PAD + 1)
        phi[s] -= inv[a:b].sum()
    phi48 = np.zeros((128, 1), np.float32)
    phi48[:24, 0] = phi[:24]
    phi48[64:88, 0] = phi[-24:]

    ident = np.eye(128, dtype=np.float32)

    # g-map: g~[k] = w~[k-12] - (1/25) sum_{|e|<=12} w~[k-12-e], as lhsT [64, 88]
    MgT = np.zeros((64, 88), np.float64)
    for d in range(64):
        for k in range(88):
            delta = k - 12
            v = 1.0 if d == delta else 0.0
            if abs(delta - d) <= 12:
                v -= 1.0 / 25.0
            MgT[d, k] = v

    # boundary-correction E as lhsT [128, 24]: ey[u'] = sum_s E[s,u'] y[s]
    # (head y rows 0..23 at partitions 0..23, tail rows L-24..L-1 at 64..87)
    Econ = np.zeros((128, 24), np.float64)
    for up in range(24):
        u = up if up < 12 else L - 24 + up
        row = np.zeros(L)
        for e in range(-PAD, PAD + 1):
            row[(u + e) % L] += 1.0 / 25.0
        a = max(0, u - PAD)
        b = min(L, u + PAD + 1)
        row[a:b] -= 1.0 / (b - a)
        for s in np.nonzero(row)[0]:
            if s < 24:
                Econ[s, up] = row[s]
            elif s >= L - 24:
                Econ[64 + s - (L - 24), up] = row[s]
            else:
                raise AssertionError((up, s))

    rev = np.zeros((64, 64), np.float32)
    for i in range(64):
        rev[i, 63 - i] = 1.0
    return phi48, ident, MgT.astype(np.float32), Econ.astype(np.float32), rev


def _build():
    import concourse.bass as bass
    import concourse.tile as tile
    import concourse.mybir as mybir
    from concourse import bacc
    import bass_rust
    import ml_dtypes

    dt = mybir.dt
    f32 = dt.float32
    bf16 = dt.bfloat16
    AF = mybir.ActivationFunctionType
    ALU = mybir.AluOpType
    AX = mybir.AxisListType
    ts = bass.ts

    nc = bacc.Bacc(None, target_bir_lowering=False)

    xe = nc.dram_tensor("xb", [L, D], f32, kind="ExternalInput")
    wqe = nc.dram_tensor("Wq", [D, D], f32, kind="ExternalInput")
    wke = nc.dram_tensor("Wk", [D, D], f32, kind="ExternalInput")
    wve = nc.dram_tensor("Wv", [D, D], f32, kind="ExternalInput")
    woe = nc.dram_tensor("Wo", [D, D], f32, kind="ExternalInput")
    bqe = nc.dram_tensor("bq", [D], f32, kind="ExternalInput")
    bke = nc.dram_tensor("bk", [D], f32, kind="ExternalInput")
    bve = nc.dram_tensor("bv", [D], f32, kind="ExternalInput")
    boe = nc.dram_tensor("bo", [D], f32, kind="ExternalInput")
    gme = nc.dram_tensor("gamma", [D], f32, kind="ExternalInput")
    oute = nc.dram_tensor("out", [L, D], f32, kind="ExternalOutput")

    phi48, ident, MgT, Econ, rev = _np_consts()
    bf = ml_dtypes.bfloat16
    cphi = nc.inline_tensor(phi48.astype(bf), "c_phi")
    cid = nc.inline_tensor(ident.astype(bf), "c_id")
    cMgT = nc.inline_tensor(MgT.astype(bf), "c_MgT")
    cEcon = nc.inline_tensor(Econ.astype(bf), "c_Econ")
    crev = nc.inline_tensor(rev.astype(bf), "c_rev")
    cones = nc.inline_tensor(np.ones((1, 128), bf), "c_o128b")
    cones64 = nc.inline_tensor(np.ones((1, 64), np.float32), "c_o64")

    from contextlib import ExitStack

    with tile.TileContext(nc) as tc, ExitStack() as ctx:
        pc = ctx.enter_context(tc.tile_pool(name="consts", bufs=1))
        px = ctx.enter_context(tc.tile_pool(name="xpairs", bufs=NP))
        pz = ctx.enter_context(tc.tile_pool(name="ztiles", bufs=10))
        pzT = ctx.enter_context(tc.tile_pool(name="zT", bufs=3))
        py = ctx.enter_context(tc.tile_pool(name="ytiles", bufs=NT))
        pwstg = ctx.enter_context(tc.tile_pool(name="wstage", bufs=2))
        pwb = ctx.enter_context(tc.tile_pool(name="wbf", bufs=1))
        pwvo = ctx.enter_context(tc.tile_pool(name="wvo", bufs=4))
        pstt = ctx.enter_context(tc.tile_pool(name="stats", bufs=3))
        psm = ctx.enter_context(tc.tile_pool(name="smalls", bufs=1))
        pout = ctx.enter_context(tc.tile_pool(name="osb", bufs=3))
        pdram = ctx.enter_context(tc.tile_pool(name="dram", bufs=1, space="DRAM"))
        q_tp = ctx.enter_context(tc.tile_pool(name="ps_tp", bufs=1, space="PSUM"))
        q_y = ctx.enter_context(tc.tile_pool(name="ps_y", bufs=1, space="PSUM"))
        q_big = ctx.enter_context(tc.tile_pool(name="ps_big", bufs=2, space="PSUM"))
        q_corr = ctx.enter_context(tc.tile_pool(name="ps_cr", bufs=1, space="PSUM"))
        q_sm = ctx.enter_context(tc.tile_pool(name="ps_sm", bufs=2, space="PSUM"))

        # ---------------- DRAM scratch ----------------
        canv_g = pdram.tile([128, 384], bf16, tag="canv_g")
        canv_c = pdram.tile([24, 192], bf16, tag="canv_c")
        g88d = pdram.tile([88], bf16, tag="g88d")
        wrevd = pdram.tile([64], bf16, tag="wrevd")
        ccin = pdram.tile([64], f32, tag="ccin")
        ccout = pdram.tile([8, 64], f32, tag="ccout")
        mvd = pdram.tile([64], f32, tag="mvd")

        # ---------------- stats path (high priority) ----------------
        with tc.high_priority():
            x48 = pc.tile([128, 512], f32, tag="x48")
            nc.sync.dma_start(x48[0:24, :], xe[0:24, :])
            nc.sync.dma_start(x48[64:88, :], xe[L - 24 : L, :])
            phis = pc.tile([128, 1], bf16, tag="phis")
            nc.sync.dma_start(phis[:], cphi[:, :])
            idt = pc.tile([128, 128], bf16, tag="idt")
            nc.sync.dma_start(idt[:], cid[:, :])
            gammaP = pc.tile([128, 4], f32, tag="gammaP")
            nc.sync.dma_start(gammaP[:], gme[:].rearrange("(a b) -> b a", b=128))

            # LN for the 48 boundary rows via bn_stats
            bs48 = pstt.tile([128, 8], f32, tag="bs48")
            nc.vector.bn_stats(bs48[0:88, 0:6], x48[0:88, :])
            nc.vector.bn_aggr(bs48[0:88, 6:8], bs48[0:88, 0:6])
            st8 = pstt.tile([128, 4], f32, tag="st48")
            nc.vector.tensor_scalar(
                st8[0:88, 0:1], bs48[0:88, 7:8], 1.0, EPS, op0=ALU.mult, op1=ALU.add
            )
            nc.scalar.activation(st8[0:88, 1:2], st8[0:88, 0:1], AF.Sqrt)
            nc.vector.reciprocal(st8[0:88, 2:3], st8[0:88, 1:2])
            nc.vector.tensor_tensor(
                st8[0:88, 3:4], bs48[0:88, 6:7], st8[0:88, 2:3], op=ALU.mult
            )
            nc.vector.tensor_scalar(
                st8[0:88, 3:4], st8[0:88, 3:4], -1.0, None, op0=ALU.mult
            )
            z48 = pc.tile([128, 512], bf16, tag="z48")
            nc.scalar.activation(
                z48[0:88, :], x48[0:88, :], AF.Identity,
                bias=st8[0:88, 3:4], scale=st8[0:88, 2:3],
            )
            qps = q_sm.tile([128, 8], f32, tag="sm")
            for c in range(4):
                nc.tensor.matmul(
                    qps[:, c : c + 1], z48[0:24, ts(c, 128)], phis[0:24, :],
                    start=True, stop=True,
                )
            for c in range(4):
                nc.tensor.matmul(
                    qps[:, 4 + c : 5 + c], z48[64:88, ts(c, 128)], phis[64:88, :],
                    start=True, stop=True,
                )

            # Wq/Wk: stage f32 (scalar queue), cast bf16 (vector)
            wqstg = pwstg.tile([128, 2048], f32, tag="wstg")
            nc.scalar.dma_start(wqstg[:].rearrange("p (a d) -> p a d", a=4), wqe[:, :].rearrange("(a p) d -> p a d", a=4))
            wqb = pwb.tile([128, 2048], bf16, tag="wqb")
            nc.vector.tensor_copy(wqb[:], wqstg[:])
            wkstg = pwstg.tile([128, 2048], f32, tag="wstg")
            nc.scalar.dma_start(wkstg[:].rearrange("p (a d) -> p a d", a=4), wke[:, :].rearrange("(a p) d -> p a d", a=4))
            wkb = pwb.tile([128, 2048], bf16, tag="wkb")
            nc.vector.tensor_copy(wkb[:], wkstg[:])

            bqv = pc.tile([1, 512], f32, tag="bqv")
            nc.sync.dma_start(bqv[:], bqe[:])
            bkv = pc.tile([1, 512], f32, tag="bkv")
            nc.sync.dma_start(bkv[:], bke[:])
            bq_sc = pc.tile([1, 512], f32, tag="bq_sc")
            nc.scalar.mul(bq_sc[:], bqv[:], float(L))
            bk_sc = pc.tile([1, 512], f32, tag="bk_sc")
            nc.scalar.mul(bk_sc[:], bkv[:], float(L))

            # qs_g = gamma * (head + tail column sums of seasonal)
            qs_t = psm.tile([128, 4], f32, tag="qst")
            nc.scalar.copy(qs_t[:], qps[:, 4:8])
            qs_g = psm.tile([128, 4], bf16, tag="qsg")
            nc.vector.tensor_tensor(qs_g[:], qps[:, 0:4], qs_t[:], op=ALU.add)
            gb = psm.tile([128, 4], bf16, tag="gb")
            nc.vector.tensor_copy(gb[:], gammaP[:])
            nc.vector.tensor_tensor(qs_g[:], qs_g[:], gb[:], op=ALU.mult)

            qs_ps = q_sm.tile([1, 512], f32, tag="sm")
            for c in range(4):
                nc.tensor.matmul(
                    qs_ps[:], qs_g[:, c : c + 1], wqb[:, ts(c, 512)],
                    start=(c == 0), stop=(c == 3),
                )
            qsv = psm.tile([1, 512], f32, tag="qsv")
            nc.vector.tensor_tensor(qsv[:], qs_ps[:], bq_sc[:], op=ALU.add)
            ks_ps = q_sm.tile([1, 512], f32, tag="sm")
            for c in range(4):
                nc.tensor.matmul(
                    ks_ps[:], qs_g[:, c : c + 1], wkb[:, ts(c, 512)],
                    start=(c == 0), stop=(c == 3),
                )
            ksv = psm.tile([1, 512], f32, tag="ksv")
            nc.vector.tensor_tensor(ksv[:], ks_ps[:], bk_sc[:], op=ALU.add)

            pr = psm.tile([1, 512], f32, tag="pr")
            nc.vector.tensor_tensor(pr[:], qsv[:], ksv[:], op=ALU.mult)
            mvr = psm.tile([1, 64], f32, tag="mvr")
            nc.vector.tensor_reduce(
                mvr[:], pr[:].rearrange("p (h c) -> p c h", h=H),
                axis=AX.X, op=ALU.add,
            )
            mv = psm.tile([1, 64], f32, tag="mv")
            nc.scalar.mul(mv[:], mvr[:], 1.0 / HL)

        # ---------------- zero toeplitz canvases, early ----------------
        zline = pc.tile([128, 384], bf16, tag="zline")
        nc.vector.memset(zline[:], 0.0)
        nc.gpsimd.dma_start(canv_g[:], zline[:])
        nc.gpsimd.dma_start(canv_c[:], zline[0:24, 0:192])

        # ---------------- x pair loads ----------------
        xp = [None] * NP

        def load_pair(k, eng):
            xt = px.tile([128, 1024], f32, tag="xp")
            eng.dma_start(
                xt[:].rearrange("p (c d) -> p c d", c=2),
                xe[256 * k : 256 * (k + 1), :].rearrange("(c p) d -> p c d", c=2),
            )
            xp[k] = xt

        # wv/wo staging on sync (after x48), interleaved with first x pairs
        load_pair(0, nc.sync)
        wvstg = pwstg.tile([128, 2048], f32, tag="wstg2")
        nc.sync.dma_start(wvstg[:].rearrange("p (a d) -> p a d", a=4), wve[:, :].rearrange("(a p) d -> p a d", a=4))
        load_pair(1, nc.sync)
        wostg = pwstg.tile([128, 2048], f32, tag="wstg2")
        nc.sync.dma_start(wostg[:].rearrange("p (a d) -> p a d", a=4), woe[:, :].rearrange("(a p) d -> p a d", a=4))
        load_pair(2, nc.sync)
        load_pair(3, nc.sync)
        for k in range(4, 10):
            load_pair(k, nc.gpsimd)
        load_pair(10, nc.scalar)
        load_pair(11, nc.scalar)

        # collective launch: on the gpsimd queue after the x-pair issues so the
        # mv semaphore wait doesn't block them; mvd written early for readback
        nc.gpsimd.dma_start(ccin[:], mv[:])
        nc.gpsimd.collective_compute(
            "AllGather",
            ALU.bypass,
            replica_groups=[list(range(NCORES))],
            ins=[ccin[:].opt()],
            outs=[ccout[:].opt()],
        )
        nc.gpsimd.dma_start(mvd[:], mv[:])

        bvP = pc.tile([128, 4], f32, tag="bvP")
        nc.sync.dma_start(bvP[:], bve[:].rearrange("(a b) -> b a", b=128))
        bov = pc.tile([1, 512], f32, tag="bov")
        nc.sync.dma_start(bov[:], boe[:])

        # ---------------- weight prep: Wvo = diag(gamma) Wv Wo ----------------
        wvs = []
        for a in range(4):
            w = pwb.tile([128, 512], bf16, tag=f"wvs{a}")
            nc.scalar.activation(
                w[:], wvstg[:, ts(a, 512)], AF.Identity, scale=gammaP[:, a : a + 1]
            )
            wvs.append(w)
        wob = pwb.tile([128, 2048], bf16, tag="wob")
        nc.vector.tensor_copy(wob[:], wostg[:])
        wvT = []
        for c in range(4):
            w = pwb.tile([128, 512], bf16, tag=f"wvT{c}")
            wvT.append(w)
        for a in range(4):
            tp = q_tp.tile([128, 512], bf16, tag="tp")
            for c in range(4):
                nc.tensor.transpose(tp[:, ts(c, 128)], wvs[a][:, ts(c, 128)], idt[:])
            for c in range(4):
                nc.vector.tensor_copy(wvT[c][:, ts(a, 128)], tp[:, ts(c, 128)])
        wvo = []
        for a in range(4):
            vps = q_y.tile([128, 512], f32, tag="y")
            for c in range(4):
                nc.tensor.matmul(
                    vps[:], wvT[c][:, ts(a, 128)], wob[:, ts(c, 512)],
                    start=(c == 0), stop=(c == 3),
                )
            w = pwvo.tile([128, 512], bf16, tag="wvo")
            nc.scalar.copy(w[:], vps[:])
            wvo.append(w)

        # cvb = bv @ Wo + bo, broadcast to [128, 512]
        bvPb = psm.tile([128, 4], bf16, tag="bvPb")
        nc.vector.tensor_copy(bvPb[:], bvP[:])
        cps = q_sm.tile([1, 512], f32, tag="sm")
        for c in range(4):
            nc.tensor.matmul(
                cps[:], bvPb[:, c : c + 1], wob[:, ts(c, 512)],
                start=(c == 0), stop=(c == 3),
            )
        cv_sb = psm.tile([1, 512], f32, tag="cv")
        nc.vector.tensor_tensor(cv_sb[:], cps[:], bov[:], op=ALU.add)
        cvb = psm.tile([1, 512], bf16, tag="cvb")
        nc.vector.tensor_copy(cvb[:], cv_sb[:])
        o128 = pc.tile([1, 128], bf16, tag="o128")
        nc.sync.dma_start(o128[:], cones[:, :])
        cbps = q_big.tile([128, 512], f32, tag="big")
        nc.tensor.matmul(cbps[:], o128[:], cvb[:], start=True, stop=True)
        cvbtile = pc.tile([128, 512], f32, tag="cvbtile")
        nc.vector.tensor_copy(cvbtile[:], cbps[:])

        # more consts (needed post-collective)
        MgTsb = pc.tile([64, 88], bf16, tag="MgTsb")
        nc.sync.dma_start(MgTsb[:], cMgT[:, :])
        Econsb = pc.tile([128, 24], bf16, tag="Econsb")
        nc.sync.dma_start(Econsb[:], cEcon[:, :])
        revsb = pc.tile([64, 64], bf16, tag="revsb")
        nc.sync.dma_start(revsb[:], crev[:, :])

        # ---------------- main pipeline: LN -> zT -> y ----------------
        ztiles = [None] * NT
        ytiles = [None] * NT

        def emit_group(g):  # tiles 4g .. 4g+3 (pairs 2g, 2g+1)
            st = pstt.tile([128, 16], f32, tag="st")
            for j in range(4):
                i = 4 * g + j
                xh = xp[i // 2][:, ts(i % 2, 512)]
                bs = pstt.tile([128, 8], f32, tag="bs")
                nc.vector.bn_stats(bs[:, 0:6], xh)
                nc.vector.bn_aggr(st[:, 2 * j : 2 * j + 2], bs[:, 0:6])
            # means at cols {0,2,4,6}, vars at {1,3,5,7}
            nc.vector.tensor_scalar(
                st[:, 8:12], st[:, 1:8:2], 1.0, EPS, op0=ALU.mult, op1=ALU.add
            )
            nc.scalar.activation(st[:, 12:16], st[:, 8:12], AF.Sqrt)
            nc.vector.reciprocal(st[:, 8:12], st[:, 12:16])  # r at 8..11
            nc.vector.tensor_tensor(
                st[:, 12:16], st[:, 0:8:2], st[:, 8:12], op=ALU.mult
            )
            nc.vector.tensor_scalar(
                st[:, 12:16], st[:, 12:16], -1.0, None, op0=ALU.mult
            )  # nmr at 12..15
            for j in range(4):
                i = 4 * g + j
                xh = xp[i // 2][:, ts(i % 2, 512)]
                z = pz.tile([128, 512], bf16, tag="z")
                nc.scalar.activation(
                    z[:], xh, AF.Identity,
                    bias=st[:, 12 + j : 13 + j], scale=st[:, 8 + j : 9 + j],
                )
                ztiles[i] = z

        def emit_y(i):
            tp = q_tp.tile([128, 512], bf16, tag="tp")
            for c in range(4):
                nc.tensor.transpose(tp[:, ts(c, 128)], ztiles[i][:, ts(c, 128)], idt[:])
            sT = pzT.tile([128, 512], bf16, tag="sT")
            nc.vector.tensor_copy(sT[:], tp[:])
            vps = q_y.tile([128, 512], f32, tag="y")
            for c in range(4):
                nc.tensor.matmul(
                    vps[:], sT[:, ts(c, 128)], wvo[c][:],
                    start=(c == 0), stop=(c == 3),
                )
            yv = py.tile([128, 512], bf16, tag="y")
            nc.scalar.copy(yv[:], vps[:])
            ytiles[i] = yv

        for g in range(6):
            emit_group(g)
            for j in range(4):
                emit_y(4 * g + j)

        # residual with cvb folded in: xp[k] += cvbtile (both halves)
        for k in range(NP):
            for h in range(2):
                nc.vector.tensor_tensor(
                    xp[k][:, ts(h, 512)], xp[k][:, ts(h, 512)], cvbtile[:],
                    op=ALU.add,
                )

        # ---------------- boundary ey = E(y) ----------------
        ytail = pc.tile([128, 512], bf16, tag="ytail")
        nc.gpsimd.dma_start(ytail[64:88, :], ytiles[23][104:128, :])
        eyps = q_big.tile([128, 512], f32, tag="big")
        nc.tensor.matmul(
            eyps[0:24, :], Econsb[0:24, :], ytiles[0][0:24, :],
            start=True, stop=False,
        )
        nc.tensor.matmul(
            eyps[0:24, :], Econsb[64:88, :], ytail[64:88, :],
            start=False, stop=True,
        )
        eysb = psm.tile([24, 512], bf16, tag="eysb")
        nc.vector.tensor_copy(eysb[:], eyps[0:24, :])

        # ---------------- collective readback -> weights -> toeplitz ----------------
        g8 = psm.tile([1, 512], f32, tag="g8")
        nc.sync.dma_start(g8[:], ccout[:])
        g_row = psm.tile([1, 64], f32, tag="grow")
        nc.vector.tensor_reduce(
            g_row[:], g8[:].rearrange("p (r c) -> p c r", r=NCORES),
            axis=AX.X, op=ALU.add,
        )
        ones64 = nc.const_aps.tensor(1.0, (64, 1))
        o1x64 = pc.tile([1, 64], f32, tag="o1x64")
        nc.sync.dma_start(o1x64[:], cones64[:, :])
        gP_ps = q_sm.tile([64, 1], f32, tag="sm")
        nc.tensor.matmul(
            gP_ps[:], g_row[:], nc.const_aps.tensor(1.0, (1, 1)),
            start=True, stop=True,
        )
        gP = psm.tile([64, 1], f32, tag="gP")
        nc.vector.tensor_copy(gP[:], gP_ps[:])
        mvP = psm.tile([64, 1], f32, tag="mvP")
        nc.sync.dma_start(mvP[:], mvd[:])

        gf_ps = q_sm.tile([64, 64], f32, tag="sm")
        nc.tensor.matmul(gf_ps[:], o1x64[:], g_row[:], start=True, stop=True)
        sc = psm.tile([64, 8], f32, tag="scm")
        cmp = psm.tile([64, 64], f32, tag="cmp")
        nc.vector.tensor_tensor(
            cmp[:], gf_ps[:], gP[:].to_broadcast((64, 64)), op=ALU.is_gt
        )
        nc.vector.tensor_reduce(sc[:, 0:1], cmp[:], axis=AX.X, op=ALU.add)
        nc.vector.tensor_scalar(
            sc[:, 1:2], sc[:, 0:1], KTOP - 0.5, None, op0=ALU.is_lt
        )
        nc.scalar.activation(sc[:, 2:3], mvP[:], AF.Exp)
        nc.vector.tensor_tensor(sc[:, 3:4], sc[:, 2:3], sc[:, 1:2], op=ALU.mult)
        s_ps = q_sm.tile([1, 1], f32, tag="sm")
        nc.tensor.matmul(s_ps[:], sc[:, 3:4], ones64, start=True, stop=True)
        rs = psm.tile([1, 1], f32, tag="rs")
        nc.vector.reciprocal(rs[:], s_ps[:])
        rsf_ps = q_sm.tile([64, 1], f32, tag="sm")
        nc.tensor.matmul(rsf_ps[:], o1x64[:], rs[:], start=True, stop=True)
        wf = psm.tile([64, 1], f32, tag="wf")
        nc.vector.tensor_tensor(wf[:], sc[:, 3:4], rsf_ps[:], op=ALU.mult)
        wfb = psm.tile([64, 1], bf16, tag="wfb")
        nc.vector.tensor_copy(wfb[:], wf[:])

        # wrev (reversed weights) and g (88-tap fused kernel)
        wrev_ps = q_sm.tile([64, 1], f32, tag="sm")
        nc.tensor.matmul(wrev_ps[:], revsb[:], wfb[:], start=True, stop=True)
        wrevb = psm.tile([64, 1], bf16, tag="wrevb")
        nc.vector.tensor_copy(wrevb[:], wrev_ps[:])
        nc.sync.dma_start(wrevd[:], wrevb[:])
        g_ps = q_sm.tile([88, 1], f32, tag="sm")
        nc.tensor.matmul(g_ps[:], MgTsb[:], wfb[:], start=True, stop=True)
        g88b = psm.tile([88, 1], bf16, tag="g88b")
        nc.vector.tensor_copy(g88b[:], g_ps[:])
        nc.sync.dma_start(g88d[:], g88b[:])

        # scatter g into the toeplitz canvas (row p, col p+k = g~[k]);
        # A-fix region at col 332+: canv[p, p+332+k] = g~[k] for p<12
        VP = bass_rust.VecI64Pair
        dst = canv_g[:].flatten()
        dst.ap = VP([[385, 128], [1, 88]])
        src = g88d[:].flatten()
        src.ap = VP([[0, 128], [1, 88]])
        nc.sync.dma_start(dst, src)
        dstA = canv_g[0:1, 332:333].flatten()
        dstA.ap = VP([[385, 12], [1, 12]])
        srcA = g88d[:].flatten()
        srcA.ap = VP([[0, 12], [1, 12]])
        nc.sync.dma_start(dstA, srcA)
        # corr canvas: head rows at col 65+u, tail rows at col 53+m, rev weights
        dstH = canv_c[0:1, 65:66].flatten()
        dstH.ap = VP([[193, 12], [1, 64]])
        srcH = wrevd[:].flatten()
        srcH.ap = VP([[0, 12], [1, 64]])
        nc.sync.dma_start(dstH, srcH)
        dstT = canv_c[12:13, 53:54].flatten()
        dstT.ap = VP([[193, 12], [1, 64]])
        srcT = wrevd[:].flatten()
        srcT.ap = VP([[0, 12], [1, 64]])
        nc.sync.dma_start(dstT, srcT)

        canvsb = pc.tile([128, 384], bf16, tag="canvsb")
        nc.sync.dma_start(canvsb[:], canv_g[:])
        cvsb = pc.tile([24, 192], bf16, tag="cvsb")
        nc.sync.dma_start(cvsb[:], canv_c[:])
        nc.vector.tensor_tensor(
            cvsb[:, 0:64], cvsb[:, 0:64], cvsb[:, 128:192], op=ALU.add
        )

        toep = {}
        for name, c0 in (("M", 12), ("N", 140), ("A", 216)):
            tpp = q_tp.tile([128, 128], bf16, tag="tp")
            nc.tensor.transpose(tpp[:], canvsb[:, c0 : c0 + 128], idt[:])
            t_sb = pc.tile([128, 128], bf16, tag=f"toep{name}")
            nc.vector.tensor_copy(t_sb[:], tpp[:])
            toep[name] = t_sb

        corrP = q_corr.tile([128, 512], f32, tag="cr")
        nc.tensor.matmul(
            corrP[:], cvsb[:, 0:128], eysb[:], start=True, stop=True
        )

        # ---------------- taps + residual + output ----------------
        ob = None
        for i in range(NT):
            tps = q_big.tile([128, 512], f32, tag="big")
            nc.tensor.matmul(
                tps[:], toep["A"][:], ytiles[(i + NT - 1) % NT][:],
                start=True, stop=False,
            )
            nc.tensor.matmul(
                tps[:], toep["M"][:], ytiles[i][:], start=False, stop=False
            )
            nc.tensor.matmul(
                tps[:], toep["N"][:], ytiles[(i + 1) % NT][:],
                start=False, stop=True,
            )
            k, hh = i // 2, i % 2
            if hh == 0:
                ob = pout.tile([128, 1024], f32, tag="ob")
            nc.vector.tensor_tensor(
                ob[:, ts(hh, 512)], xp[k][:, ts(hh, 512)], tps[:], op=ALU.add
            )
            if i == 0:
                nc.vector.tensor_tensor(
                    ob[0:32, 0:512], ob[0:32, 0:512], corrP[0:32, :], op=ALU.add
                )
            if i == 23:
                nc.vector.tensor_tensor(
                    ob[32:64, 512:1024], ob[32:64, 512:1024], corrP[32:64, :],
                    op=ALU.add,
                )
                nc.vector.tensor_tensor(
                    ob[64:128, 512:1024], ob[64:128, 512:1024], corrP[64:128, :],
                    op=ALU.add,
                )
            if hh == 1:
                eng = nc.scalar if (k % 2 == 0) else nc.sync
                eng.dma_start(
                    oute[256 * k : 256 * (k + 1), :].rearrange(
                        "(c p) d -> p c d", c=2
                    ),
                    ob[:].rearrange("p (c d) -> p c d", c=2),
                )

    nc.finalize()
    return nc


def _get_nc():
    if "nc" not in _CACHE:
        _CACHE["nc"] = _build()
    return _CACHE["nc"]


def kernel_ext(inputs, trace=False):
    from concourse.bass_utils import run_bass_kernel_spmd

    nc = _get_nc()
    x = np.ascontiguousarray(inputs["x"], np.float32)
    common = {
        k: np.ascontiguousarray(inputs[k], np.float32)
        for k in ["Wq", "Wk", "Wv", "Wo", "bq", "bk", "bv", "bo", "gamma"]
    }
    in_maps = [{"xb": x[i], **common} for i in range(NCORES)]
    res = run_bass_kernel_spmd(nc, in_maps, list(range(NCORES)), trace=trace)
    out = np.stack([res.results[i]["out"] for i in range(NCORES)], axis=0)
    return out, res


def kernel(**inputs):
    out, _ = kernel_ext(inputs)
    return out
